# revision 9
# baseline (speedup 1.0000x reference)
"""BoxBlur 13x13 depthwise conv (reflect pad) on 8 trn2 NeuronCores.

Input (8, 64, 512, 512) f32 + kernel (1, 13, 13) f32 -> output (8, 64, 512, 512).

Sharding: batch dim across 8 cores (one sample = 64 channel-images per core).

Algorithm (per 512x512 image): box blur is separable. Both 1D 13-tap passes
(reflect padding folded into an integer band matrix M[h, h'] built on host)
run on the tensor engine as normal-mode matmuls with the image block as the
STATIONARY operand and the band matrix as the MOVING operand, which fuses a
transpose into each pass:

    pass1:  Y1t[w, h'] = sum_h X[h, w] * M[h, h']      (vconv, output transposed)
    pass2:  out[h', w'] = sum_w Y1t[w, h'] * M2[w, w'] (hconv, transpose undone)

Each pass is 4 contraction blocks x 4 stationary 128-slices = 16 matmuls per
image, PSUM-accumulated over the contraction blocks using partial-range
windows (the band is zero outside a ~140-wide window per block).

Default mode "f16": everything fp16 (the rel-err budget is 2e-2; fp16
end-to-end lands ~3e-4). This halves HBM traffic (the bottleneck) vs f32 and
runs every matmul single-pass at 1 cyc/row. The host converts in/out; the
1/169 scale is folded into the pass-2 band so every PSUM evacuation is a
plain copy. IO is one batched DMA per image per direction ([128, 2048]
slabs via rearranged access patterns) to stay under the SP DGE issue rate,
and PSUM is split into 2-bank tiles so evacuations are 2 big copies per pass
balanced across ACT and DVE.

Legacy modes kept for reference: f32 / mixed / f32r (see _build_nc).
"""
import numpy as np

B, C, H, W = 8, 64, 512, 512
KY = KX = 13
HALF = 6
N_CORES = 8
P = 128
NBLK = H // P  # 4

# per contraction block k: window [start, width) of nonzero band columns
_WINDOWS = [
    (max(0, P * k - HALF),
     min(H, P * k + P - 1 + HALF + 1) - max(0, P * k - HALF))
    for k in range(NBLK)
]
# fp32r runs at 1 cyc/row only when the moving free dim is >= 256: use
# widened 256-col windows (zero band entries outside the true span are
# harmless -- PSUM accumulate/overwrite-by-has_written keeps them exact)
_WINDOWS_F32R = [
    (min(max(0, P * k - HALF), H - 256), 256)
    for k in range(NBLK)
]


def _band_matrix() -> np.ndarray:
    """M[h, h'] = number of taps of output h' that hit input row h
    (13-tap, reflect padding, pad = 6 both sides)."""
    m = np.zeros((H, H), dtype=np.float32)
    for hp in range(H):
        for d in range(-HALF, HALF + 1):
            h = hp + d
            if h < 0:
                h = -h
            if h > H - 1:
                h = 2 * (H - 1) - h
            m[h, hp] += 1.0
    return m


def _build_nc_f16(n_images: int):
    import concourse.bacc as bacc
    import concourse.mybir as mybir
    from concourse.tile import TileContext

    f16 = mybir.dt.float16
    f32 = mybir.dt.float32
    nc = bacc.Bacc(trn_type="TRN2")

    x = nc.dram_tensor("x", [n_images, H, W], f16, kind="ExternalInput")
    band1 = [
        nc.dram_tensor(f"band1_{k}", [P, _WINDOWS[k][1]], f16,
                       kind="ExternalInput")
        for k in range(NBLK)
    ]
    band2 = [
        nc.dram_tensor(f"band2_{k}", [P, _WINDOWS[k][1]], f16,
                       kind="ExternalInput")
        for k in range(NBLK)
    ]
    y = nc.dram_tensor("y", [n_images, H, W], f16, kind="ExternalOutput")

    with TileContext(nc) as tc:
        with (
            tc.tile_pool(name="const", bufs=1) as const_pool,
            tc.tile_pool(name="xin", bufs=10) as x_pool,
            tc.tile_pool(name="mid", bufs=4) as mid_pool,
            tc.tile_pool(name="yout", bufs=6) as y_pool,
            tc.tile_pool(name="ps1", bufs=1, space="PSUM") as ps1_pool,
            tc.tile_pool(name="ps2", bufs=1, space="PSUM") as ps2_pool,
        ):
            b1, b2 = [], []
            for k in range(NBLK):
                t1 = const_pool.tile([P, _WINDOWS[k][1]], f16, tag=f"b1{k}")
                nc.sync.dma_start(t1[:], band1[k][:])
                b1.append(t1)
                t2 = const_pool.tile([P, _WINDOWS[k][1]], f16, tag=f"b2{k}")
                nc.sync.dma_start(t2[:], band2[k][:])
                b2.append(t2)

            def pass1(c):
                # one DMA per image: [128, 4*512] slab, cols = strip k | w
                xt = x_pool.tile([P, NBLK * W], f16, tag="xt")
                nc.sync.dma_start(
                    xt[:].rearrange("p (k w) -> p k w", w=W),
                    x[c, :, :].rearrange("(k p) w -> p k w", p=P))

                # per half = two j-slices into one 2-bank PSUM tile
                mids = []
                for half in range(2):
                    ps = ps1_pool.tile([P, 2 * H], f32, tag=f"ps1{half}")
                    for jj in range(2):
                        j = 2 * half + jj
                        for k in range(NBLK):
                            w0, wid = _WINDOWS[k]
                            nc.tensor.matmul(
                                ps[:, jj * H + w0:jj * H + w0 + wid],
                                xt[:, k * W + P * j:k * W + P * (j + 1)],
                                b1[k][:],
                                start=(k == 0), stop=(k == NBLK - 1),
                            )
                    mt = mid_pool.tile([P, 2 * H], f16, tag=f"mid{half}")
                    if half == 0:
                        nc.scalar.copy(mt[:], ps[:])
                    else:
                        nc.vector.tensor_copy(mt[:], ps[:])
                    mids.append(mt)
                return mids

            def pass2(c, mids):
                # per half = two i-slices into one 2-bank PSUM tile
                yt = y_pool.tile([P, NBLK * W], f16, tag="yt")
                for half in range(2):
                    ps = ps2_pool.tile([P, 2 * W], f32, tag=f"ps2{half}")
                    for ii in range(2):
                        i = 2 * half + ii
                        for j in range(NBLK):
                            w0, wid = _WINDOWS[j]
                            nc.tensor.matmul(
                                ps[:, ii * W + w0:ii * W + w0 + wid],
                                mids[j // 2][:, (j % 2) * H + P * i:
                                             (j % 2) * H + P * (i + 1)],
                                b2[j][:],
                                start=(j == 0), stop=(j == NBLK - 1),
                            )
                    if half == 0:
                        nc.vector.tensor_copy(
                            yt[:, half * 2 * W:(half + 1) * 2 * W], ps[:])
                    else:
                        nc.scalar.copy(
                            yt[:, half * 2 * W:(half + 1) * 2 * W], ps[:])
                # SWDGE (GpSimd) queue: keeps output stores off the SP HWDGE
                # FIFO so they never head-of-line-block input prefetch
                nc.gpsimd.dma_start(
                    y[c, :, :].rearrange("(i p) w -> p i w", p=P),
                    yt[:].rearrange("p (i w) -> p i w", w=W))

            # software pipeline: pass2 runs one image behind pass1, so the
            # PE fills the mid-evacuation latency of image c with pass-1
            # matmuls of image c+1 instead of stalling
            mids_prev = None
            for c in range(n_images):
                mids_c = pass1(c)
                if mids_prev is not None:
                    pass2(c - 1, mids_prev)
                mids_prev = mids_c
            pass2(n_images - 1, mids_prev)

    nc.compile()
    return nc


def _build_nc_i8(n_images: int):
    """int8-input variant: input DMA'd as int8 (halves input HBM traffic),
    upcast to f16 on the GpSimd engine (otherwise idle), dequant scale folded
    into the pass-1 band. Output stays f16. IO DMAs on the SP HWDGE queue;
    GpSimd is dedicated to the casts."""
    import concourse.bacc as bacc
    import concourse.mybir as mybir
    from concourse.tile import TileContext

    i8 = mybir.dt.int8
    f16 = mybir.dt.float16
    f32 = mybir.dt.float32
    nc = bacc.Bacc(trn_type="TRN2")

    x = nc.dram_tensor("x", [n_images, H, W], i8, kind="ExternalInput")
    band1 = [
        nc.dram_tensor(f"band1_{k}", [P, _WINDOWS[k][1]], f16,
                       kind="ExternalInput")
        for k in range(NBLK)
    ]
    band2 = [
        nc.dram_tensor(f"band2_{k}", [P, _WINDOWS[k][1]], f16,
                       kind="ExternalInput")
        for k in range(NBLK)
    ]
    y = nc.dram_tensor("y", [n_images, H, W], f16, kind="ExternalOutput")

    with TileContext(nc) as tc:
        with (
            tc.tile_pool(name="const", bufs=1) as const_pool,
            tc.tile_pool(name="xq", bufs=10) as xq_pool,
            tc.tile_pool(name="xin", bufs=4) as x_pool,
            tc.tile_pool(name="mid", bufs=4) as mid_pool,
            tc.tile_pool(name="yout", bufs=6) as y_pool,
            tc.tile_pool(name="ps1", bufs=1, space="PSUM") as ps1_pool,
            tc.tile_pool(name="ps2", bufs=1, space="PSUM") as ps2_pool,
        ):
            b1, b2 = [], []
            for k in range(NBLK):
                t1 = const_pool.tile([P, _WINDOWS[k][1]], f16, tag=f"b1{k}")
                nc.sync.dma_start(t1[:], band1[k][:])
                b1.append(t1)
                t2 = const_pool.tile([P, _WINDOWS[k][1]], f16, tag=f"b2{k}")
                nc.sync.dma_start(t2[:], band2[k][:])
                b2.append(t2)

            def pass1(c):
                xq = xq_pool.tile([P, NBLK * W], i8, tag="xq")
                nc.sync.dma_start(
                    xq[:].rearrange("p (k w) -> p k w", w=W),
                    x[c, :, :].rearrange("(k p) w -> p k w", p=P))
                # upcast int8 -> f16 on the idle GpSimd engine
                xt = x_pool.tile([P, NBLK * W], f16, tag="xt")
                nc.gpsimd.tensor_copy(xt[:], xq[:])

                mids = []
                for half in range(2):
                    ps = ps1_pool.tile([P, 2 * H], f32, tag=f"ps1{half}")
                    for jj in range(2):
                        j = 2 * half + jj
                        for k in range(NBLK):
                            w0, wid = _WINDOWS[k]
                            nc.tensor.matmul(
                                ps[:, jj * H + w0:jj * H + w0 + wid],
                                xt[:, k * W + P * j:k * W + P * (j + 1)],
                                b1[k][:],
                                start=(k == 0), stop=(k == NBLK - 1),
                            )
                    mt = mid_pool.tile([P, 2 * H], f16, tag=f"mid{half}")
                    if half == 0:
                        nc.scalar.copy(mt[:], ps[:])
                    else:
                        nc.vector.tensor_copy(mt[:], ps[:])
                    mids.append(mt)
                return mids

            def pass2(c, mids):
                yt = y_pool.tile([P, NBLK * W], f16, tag="yt")
                for half in range(2):
                    ps = ps2_pool.tile([P, 2 * W], f32, tag=f"ps2{half}")
                    for ii in range(2):
                        i = 2 * half + ii
                        for j in range(NBLK):
                            w0, wid = _WINDOWS[j]
                            nc.tensor.matmul(
                                ps[:, ii * W + w0:ii * W + w0 + wid],
                                mids[j // 2][:, (j % 2) * H + P * i:
                                             (j % 2) * H + P * (i + 1)],
                                b2[j][:],
                                start=(j == 0), stop=(j == NBLK - 1),
                            )
                    # DVE takes evac2A plus the front of evac2B; ACT the rest
                    if half == 0:
                        nc.vector.tensor_copy(
                            yt[:, 0:2 * W], ps[:])
                    else:
                        nc.vector.tensor_copy(
                            yt[:, 2 * W:2 * W + 256], ps[:, 0:256])
                        nc.scalar.copy(
                            yt[:, 2 * W + 256:4 * W], ps[:, 256:2 * W])
                nc.sync.dma_start(
                    y[c, :, :].rearrange("(i p) w -> p i w", p=P),
                    yt[:].rearrange("p (i w) -> p i w", w=W))

            mids_prev = None
            for c in range(n_images):
                mids_c = pass1(c)
                if mids_prev is not None:
                    pass2(c - 1, mids_prev)
                mids_prev = mids_c
            pass2(n_images - 1, mids_prev)

    nc.compile()
    return nc


def _run_i8(inputs: dict, trace: bool = False):
    from concourse.bass_utils import run_bass_kernel_spmd

    xf = np.asarray(inputs["input"], dtype=np.float32)
    ker = np.asarray(inputs["kernel"], dtype=np.float32)
    scale = float(ker[0, 0, 0])

    alpha = float(np.abs(xf).max()) / 127.0
    xq = np.clip(np.rint(xf * (1.0 / alpha)), -127, 127).astype(np.int8)

    m = _band_matrix()

    def win(k):
        return m[P * k:P * (k + 1),
                 _WINDOWS[k][0]:_WINDOWS[k][0] + _WINDOWS[k][1]]

    bands1 = [np.ascontiguousarray(win(k) * alpha).astype(np.float16)
              for k in range(NBLK)]
    bands2 = [np.ascontiguousarray(win(k) * scale).astype(np.float16)
              for k in range(NBLK)]

    nc = _build_nc_i8(C)
    in_maps = []
    for b in range(B):
        im = {"x": xq[b]}
        for k in range(NBLK):
            im[f"band1_{k}"] = bands1[k]
            im[f"band2_{k}"] = bands2[k]
        in_maps.append(im)

    res = run_bass_kernel_spmd(nc, in_maps, core_ids=list(range(N_CORES)),
                               trace=trace)
    out = np.stack(
        [res.results[b]["y"].astype(np.float32) for b in range(B)], axis=0)
    return out, res


def _run_f16(inputs: dict, trace: bool = False):
    from concourse.bass_utils import run_bass_kernel_spmd

    x16 = np.asarray(inputs["input"], dtype=np.float32).astype(np.float16)
    ker = np.asarray(inputs["kernel"], dtype=np.float32)
    scale = float(ker[0, 0, 0])

    m = _band_matrix()
    bands1 = [
        np.ascontiguousarray(
            m[P * k:P * (k + 1),
              _WINDOWS[k][0]:_WINDOWS[k][0] + _WINDOWS[k][1]]
        ).astype(np.float16)
        for k in range(NBLK)
    ]
    bands2 = [
        np.ascontiguousarray(
            m[P * k:P * (k + 1),
              _WINDOWS[k][0]:_WINDOWS[k][0] + _WINDOWS[k][1]] * scale
        ).astype(np.float16)
        for k in range(NBLK)
    ]

    nc = _build_nc_f16(C)
    in_maps = []
    for b in range(B):
        im = {"x": x16[b]}
        for k in range(NBLK):
            im[f"band1_{k}"] = bands1[k]
            im[f"band2_{k}"] = bands2[k]
        in_maps.append(im)

    res = run_bass_kernel_spmd(nc, in_maps, core_ids=list(range(N_CORES)),
                               trace=trace)
    out = np.stack(
        [res.results[b]["y"].astype(np.float32) for b in range(B)], axis=0)
    return out, res


# ---------------------------------------------------------------------------
# legacy f32 / mixed / f32r path (previous baseline)
# ---------------------------------------------------------------------------

def _build_nc(scale: float, n_images: int, mode: str):
    import concourse.bacc as bacc
    import concourse.mybir as mybir
    from concourse.tile import TileContext

    f32r = mode == "f32r"
    mixed = mode == "mixed"
    dt = mybir.dt.float32r if f32r else mybir.dt.float32
    wins = _WINDOWS_F32R if f32r else _WINDOWS
    nc = bacc.Bacc(trn_type="TRN2")

    x = nc.dram_tensor("x", [n_images, H, W], dt, kind="ExternalInput")
    band = [
        nc.dram_tensor(f"band{k}", [P, wins[k][1]], dt, kind="ExternalInput")
        for k in range(NBLK)
    ]
    if mixed:
        band_bf = [
            nc.dram_tensor(f"bandbf{k}", [P, wins[k][1]], mybir.dt.bfloat16,
                           kind="ExternalInput")
            for k in range(NBLK)
        ]
    y = nc.dram_tensor("y", [n_images, H, W], mybir.dt.float32,
                       kind="ExternalOutput")

    with TileContext(nc) as tc:
        with (
            tc.tile_pool(name="const", bufs=1) as const_pool,
            tc.tile_pool(name="xin", bufs=12) as x_pool,
            tc.tile_pool(name="mid", bufs=12) as mid_pool,
            tc.tile_pool(name="oout", bufs=12) as out_pool,
            tc.tile_pool(name="ps1", bufs=4, space="PSUM") as ps1_pool,
            tc.tile_pool(name="ps2", bufs=4, space="PSUM") as ps2_pool,
        ):
            band_t = []
            for k in range(NBLK):
                bt = const_pool.tile([P, wins[k][1]], dt, tag=f"band{k}")
                nc.sync.dma_start(bt[:], band[k][:])
                band_t.append(bt)
            band_bf_t = []
            if mixed:
                for k in range(NBLK):
                    bt = const_pool.tile([P, wins[k][1]], mybir.dt.bfloat16,
                                         tag=f"bandbf{k}")
                    nc.sync.dma_start(bt[:], band_bf[k][:])
                    band_bf_t.append(bt)

            def pass1(c):
                # load image as 4 row-strips
                xs = []
                for k in range(NBLK):
                    xt = x_pool.tile([P, W], dt)
                    nc.sync.dma_start(xt[:], x[c, P * k:P * (k + 1), :])
                    xs.append(xt)

                # pass 1: Y1t_j[w, h'] = sum_h X[h, 128j + w] M[h, h']
                y1 = []
                for j in range(NBLK):
                    ps = ps1_pool.tile([P, H], mybir.dt.float32)
                    for k in range(NBLK):
                        w0, wid = wins[k]
                        nc.tensor.matmul(
                            ps[:, w0:w0 + wid],
                            xs[k][:, P * j:P * (j + 1)],
                            band_t[k][:],
                            start=(k == 0), stop=(k == NBLK - 1),
                        )
                    if mixed:
                        # evacuate as bf16 hi + bf16 lo (exact to ~2^-18);
                        # bank 0 chains hi+lo on DVE (no cross-engine hop) so
                        # pass2 can start earliest; other banks' hi go to ACT
                        hi = mid_pool.tile([P, H], mybir.dt.bfloat16, tag="hi")
                        lo = mid_pool.tile([P, H], mybir.dt.bfloat16, tag="lo")
                        if j == 0:
                            nc.vector.tensor_copy(hi[:], ps[:])
                        else:
                            nc.scalar.copy(hi[:], ps[:])
                        nc.vector.tensor_sub(lo[:], ps[:], hi[:])
                        y1.append((hi, lo))
                    else:
                        yt = mid_pool.tile([P, H], dt)
                        if j % 2 == 0:
                            nc.vector.tensor_copy(yt[:], ps[:])
                        else:
                            nc.scalar.copy(yt[:], ps[:])
                        y1.append(yt)
                return y1

            def pass2(c, y1):
                # pass 2: out_i[h', w'] = sum_w Y1t[w, 128i + h'] M[w, w']
                for i in range(NBLK):
                    ps = ps2_pool.tile([P, W], mybir.dt.float32, name="ps2",
                                       tag="ps2")
                    for j in range(NBLK):
                        w0, wid = wins[j]
                        if mixed:
                            hi, lo = y1[j]
                            nc.tensor.matmul(
                                ps[:, w0:w0 + wid],
                                hi[:, P * i:P * (i + 1)],
                                band_bf_t[j][:],
                                start=(j == 0), stop=False,
                            )
                            nc.tensor.matmul(
                                ps[:, w0:w0 + wid],
                                lo[:, P * i:P * (i + 1)],
                                band_bf_t[j][:],
                                start=False, stop=(j == NBLK - 1),
                            )
                        else:
                            nc.tensor.matmul(
                                ps[:, w0:w0 + wid],
                                y1[j][:, P * i:P * (i + 1)],
                                band_t[j][:],
                                start=(j == 0), stop=(j == NBLK - 1),
                            )
                    ot = out_pool.tile([P, W], mybir.dt.float32)
                    if (not mixed and i % 2 == 0) or (mixed and i == 0):
                        nc.vector.tensor_scalar_mul(ot[:], ps[:], scale)
                    else:
                        nc.scalar.mul(ot[:], ps[:], scale)
                    nc.sync.dma_start(y[c, P * i:P * (i + 1), :], ot[:])

            for c in range(n_images):
                pass2(c, pass1(c))

    nc.compile()
    return nc


def _run(inputs: dict, mode: str = "f16", trace: bool = False):
    if mode == "f16":
        return _run_f16(inputs, trace=trace)
    if mode == "i8":
        return _run_i8(inputs, trace=trace)

    import ml_dtypes
    from concourse.bass_utils import run_bass_kernel_spmd

    x = np.ascontiguousarray(inputs["input"], dtype=np.float32)
    ker = np.asarray(inputs["kernel"], dtype=np.float32)
    scale = float(ker[0, 0, 0])

    wins = _WINDOWS_F32R if mode == "f32r" else _WINDOWS
    m = _band_matrix()
    bands = [
        np.ascontiguousarray(m[P * k:P * (k + 1), wins[k][0]:wins[k][0] + wins[k][1]])
        for k in range(NBLK)
    ]

    nc = _build_nc(scale, C, mode)
    in_maps = []
    for b in range(B):
        im = {"x": x[b]}
        for k in range(NBLK):
            im[f"band{k}"] = bands[k]
            if mode == "mixed":
                im[f"bandbf{k}"] = bands[k].astype(ml_dtypes.bfloat16)
        in_maps.append(im)

    res = run_bass_kernel_spmd(nc, in_maps, core_ids=list(range(N_CORES)),
                               trace=trace)
    out = np.stack([res.results[b]["y"] for b in range(B)], axis=0)
    return out, res


def kernel(**inputs) -> np.ndarray:
    out, _ = _run(inputs)
    return out


# revision 11
# speedup vs baseline: 2.0075x; 2.0075x over previous
"""BoxBlur 13x13 depthwise conv (reflect pad) on 8 trn2 NeuronCores.

Input (8, 64, 512, 512) f32 + kernel (1, 13, 13) f32 -> output (8, 64, 512, 512).

Sharding: batch dim across 8 cores (one sample = 64 channel-images per core).

Algorithm (per 512x512 image): box blur is separable. Both 1D 13-tap passes
(reflect padding folded into an integer band matrix M[h, h'] built on host)
run on the tensor engine as normal-mode matmuls with the image block as the
STATIONARY operand and the band matrix as the MOVING operand, which fuses a
transpose into each pass:

    pass1:  Y1t[w, h'] = sum_h X[h, w] * M[h, h']      (vconv, output transposed)
    pass2:  out[h', w'] = sum_w Y1t[w, h'] * M2[w, w'] (hconv, transpose undone)

Each pass is 4 contraction blocks x 4 stationary 128-slices = 16 matmuls per
image, PSUM-accumulated over the contraction blocks using partial-range
windows (the band is zero outside a ~140-wide window per block).

Default mode "f16": everything fp16 (the rel-err budget is 2e-2; fp16
end-to-end lands ~3e-4). This halves HBM traffic (the bottleneck) vs f32 and
runs every matmul single-pass at 1 cyc/row. The host converts in/out; the
1/169 scale is folded into the pass-2 band so every PSUM evacuation is a
plain copy. IO is one batched DMA per image per direction ([128, 2048]
slabs via rearranged access patterns) to stay under the SP DGE issue rate,
and PSUM is split into 2-bank tiles so evacuations are 2 big copies per pass
balanced across ACT and DVE.

Legacy modes kept for reference: f32 / mixed / f32r (see _build_nc).
"""
import numpy as np

B, C, H, W = 8, 64, 512, 512
KY = KX = 13
HALF = 6
N_CORES = 8
P = 128
NBLK = H // P  # 4

# per contraction block k: window [start, width) of nonzero band columns
_WINDOWS = [
    (max(0, P * k - HALF),
     min(H, P * k + P - 1 + HALF + 1) - max(0, P * k - HALF))
    for k in range(NBLK)
]
# fp32r runs at 1 cyc/row only when the moving free dim is >= 256: use
# widened 256-col windows (zero band entries outside the true span are
# harmless -- PSUM accumulate/overwrite-by-has_written keeps them exact)
_WINDOWS_F32R = [
    (min(max(0, P * k - HALF), H - 256), 256)
    for k in range(NBLK)
]


def _band_matrix() -> np.ndarray:
    """M[h, h'] = number of taps of output h' that hit input row h
    (13-tap, reflect padding, pad = 6 both sides)."""
    m = np.zeros((H, H), dtype=np.float32)
    for hp in range(H):
        for d in range(-HALF, HALF + 1):
            h = hp + d
            if h < 0:
                h = -h
            if h > H - 1:
                h = 2 * (H - 1) - h
            m[h, hp] += 1.0
    return m


def _build_nc_f16(n_images: int):
    import concourse.bacc as bacc
    import concourse.mybir as mybir
    from concourse.tile import TileContext

    f16 = mybir.dt.float16
    f32 = mybir.dt.float32
    nc = bacc.Bacc(trn_type="TRN2")

    x = nc.dram_tensor("x", [n_images, H, W], f16, kind="ExternalInput")
    band1 = [
        nc.dram_tensor(f"band1_{k}", [P, _WINDOWS[k][1]], f16,
                       kind="ExternalInput")
        for k in range(NBLK)
    ]
    band2 = [
        nc.dram_tensor(f"band2_{k}", [P, _WINDOWS[k][1]], f16,
                       kind="ExternalInput")
        for k in range(NBLK)
    ]
    y = nc.dram_tensor("y", [n_images, H, W], f16, kind="ExternalOutput")

    with TileContext(nc) as tc:
        with (
            tc.tile_pool(name="const", bufs=1) as const_pool,
            tc.tile_pool(name="xin", bufs=4) as x_pool,
            tc.tile_pool(name="mid", bufs=4) as mid_pool,
            tc.tile_pool(name="yout", bufs=3) as y_pool,
            tc.tile_pool(name="ps1", bufs=1, space="PSUM") as ps1_pool,
            tc.tile_pool(name="ps2", bufs=1, space="PSUM") as ps2_pool,
        ):
            b1, b2 = [], []
            for k in range(NBLK):
                t1 = const_pool.tile([P, _WINDOWS[k][1]], f16, tag=f"b1{k}")
                nc.sync.dma_start(t1[:], band1[k][:])
                b1.append(t1)
                t2 = const_pool.tile([P, _WINDOWS[k][1]], f16, tag=f"b2{k}")
                nc.sync.dma_start(t2[:], band2[k][:])
                b2.append(t2)

            # IO in 2-image slabs: one DMA per slab per direction halves the
            # DGE instruction rate (SP HWDGE in, GpSimd SWDGE out)
            GI = 2  # images per IO slab
            IMG = NBLK * W  # 2048 cols per image

            def pass1(c, xt):
                # per half = two j-slices into one 2-bank PSUM tile
                xoff = (c % GI) * IMG
                mids = []
                for half in range(2):
                    ps = ps1_pool.tile([P, 2 * H], f32, tag=f"ps1{half}")
                    for jj in range(2):
                        j = 2 * half + jj
                        for k in range(NBLK):
                            w0, wid = _WINDOWS[k]
                            nc.tensor.matmul(
                                ps[:, jj * H + w0:jj * H + w0 + wid],
                                xt[:, xoff + k * W + P * j:
                                   xoff + k * W + P * (j + 1)],
                                b1[k][:],
                                start=(k == 0), stop=(k == NBLK - 1),
                            )
                    mt = mid_pool.tile([P, 2 * H], f16, tag=f"mid{half}")
                    if half == 0:
                        nc.scalar.copy(mt[:], ps[:])
                    else:
                        nc.vector.tensor_copy(mt[:], ps[:])
                    mids.append(mt)
                return mids

            def pass2(c, mids, yt):
                # per half = two i-slices into one 2-bank PSUM tile
                yoff = (c % GI) * IMG
                for half in range(2):
                    ps = ps2_pool.tile([P, 2 * W], f32, tag=f"ps2{half}")
                    for ii in range(2):
                        i = 2 * half + ii
                        for j in range(NBLK):
                            w0, wid = _WINDOWS[j]
                            nc.tensor.matmul(
                                ps[:, ii * W + w0:ii * W + w0 + wid],
                                mids[j // 2][:, (j % 2) * H + P * i:
                                             (j % 2) * H + P * (i + 1)],
                                b2[j][:],
                                start=(j == 0), stop=(j == NBLK - 1),
                            )
                    if half == 0:
                        nc.vector.tensor_copy(
                            yt[:, yoff + half * 2 * W:
                               yoff + (half + 1) * 2 * W], ps[:])
                    else:
                        nc.scalar.copy(
                            yt[:, yoff + half * 2 * W:
                               yoff + (half + 1) * 2 * W], ps[:])
                if c % GI == GI - 1:
                    c0 = c - (GI - 1)
                    # SWDGE (GpSimd) queue: keeps output stores off the SP
                    # HWDGE FIFO so they never block input prefetch
                    nc.gpsimd.dma_start(
                        y[c0:c0 + GI, :, :].rearrange(
                            "n (i p) w -> p n i w", p=P),
                        yt[:].rearrange("p (n i w) -> p n i w", n=GI, w=W))

            def load_slab(c0):
                xt = x_pool.tile([P, GI * IMG], f16, tag="xt")
                nc.sync.dma_start(
                    xt[:].rearrange("p (n k w) -> p n k w", n=GI, w=W),
                    x[c0:c0 + GI, :, :].rearrange("n (k p) w -> p n k w", p=P))
                return xt

            # software pipeline: pass2 runs one image behind pass1, so the
            # PE fills the mid-evacuation latency of image c with pass-1
            # matmuls of image c+1 instead of stalling
            mids_prev = None
            xt_cur = None
            yt_cur = None
            yt_prev = None
            for c in range(n_images):
                if c % GI == 0:
                    xt_cur = load_slab(c)
                mids_c = pass1(c, xt_cur)
                if mids_prev is not None:
                    cp = c - 1
                    if cp % GI == 0:
                        yt_prev = y_pool.tile([P, GI * IMG], f16, tag="yt")
                    pass2(cp, mids_prev, yt_prev)
                mids_prev = mids_c
            cp = n_images - 1
            if cp % GI == 0:
                yt_prev = y_pool.tile([P, GI * IMG], f16, tag="yt")
            pass2(cp, mids_prev, yt_prev)

    nc.compile()
    return nc


def _build_nc_i8(n_images: int):
    """int8-input variant: input DMA'd as int8 (halves input HBM traffic),
    upcast to f16 on the GpSimd engine (otherwise idle), dequant scale folded
    into the pass-1 band. Output stays f16. IO DMAs on the SP HWDGE queue;
    GpSimd is dedicated to the casts."""
    import concourse.bacc as bacc
    import concourse.mybir as mybir
    from concourse.tile import TileContext

    i8 = mybir.dt.int8
    f16 = mybir.dt.float16
    f32 = mybir.dt.float32
    nc = bacc.Bacc(trn_type="TRN2")

    x = nc.dram_tensor("x", [n_images, H, W], i8, kind="ExternalInput")
    band1 = [
        nc.dram_tensor(f"band1_{k}", [P, _WINDOWS[k][1]], f16,
                       kind="ExternalInput")
        for k in range(NBLK)
    ]
    band2 = [
        nc.dram_tensor(f"band2_{k}", [P, _WINDOWS[k][1]], f16,
                       kind="ExternalInput")
        for k in range(NBLK)
    ]
    y = nc.dram_tensor("y", [n_images, H, W], f16, kind="ExternalOutput")

    with TileContext(nc) as tc:
        with (
            tc.tile_pool(name="const", bufs=1) as const_pool,
            tc.tile_pool(name="xq", bufs=10) as xq_pool,
            tc.tile_pool(name="xin", bufs=4) as x_pool,
            tc.tile_pool(name="mid", bufs=4) as mid_pool,
            tc.tile_pool(name="yout", bufs=6) as y_pool,
            tc.tile_pool(name="ps1", bufs=1, space="PSUM") as ps1_pool,
            tc.tile_pool(name="ps2", bufs=1, space="PSUM") as ps2_pool,
        ):
            b1, b2 = [], []
            for k in range(NBLK):
                t1 = const_pool.tile([P, _WINDOWS[k][1]], f16, tag=f"b1{k}")
                nc.sync.dma_start(t1[:], band1[k][:])
                b1.append(t1)
                t2 = const_pool.tile([P, _WINDOWS[k][1]], f16, tag=f"b2{k}")
                nc.sync.dma_start(t2[:], band2[k][:])
                b2.append(t2)

            def pass1(c):
                xq = xq_pool.tile([P, NBLK * W], i8, tag="xq")
                nc.sync.dma_start(
                    xq[:].rearrange("p (k w) -> p k w", w=W),
                    x[c, :, :].rearrange("(k p) w -> p k w", p=P))
                # upcast int8 -> f16 on the idle GpSimd engine
                xt = x_pool.tile([P, NBLK * W], f16, tag="xt")
                nc.gpsimd.tensor_copy(xt[:], xq[:])

                mids = []
                for half in range(2):
                    ps = ps1_pool.tile([P, 2 * H], f32, tag=f"ps1{half}")
                    for jj in range(2):
                        j = 2 * half + jj
                        for k in range(NBLK):
                            w0, wid = _WINDOWS[k]
                            nc.tensor.matmul(
                                ps[:, jj * H + w0:jj * H + w0 + wid],
                                xt[:, k * W + P * j:k * W + P * (j + 1)],
                                b1[k][:],
                                start=(k == 0), stop=(k == NBLK - 1),
                            )
                    mt = mid_pool.tile([P, 2 * H], f16, tag=f"mid{half}")
                    if half == 0:
                        nc.scalar.copy(mt[:], ps[:])
                    else:
                        nc.vector.tensor_copy(mt[:], ps[:])
                    mids.append(mt)
                return mids

            def pass2(c, mids):
                yt = y_pool.tile([P, NBLK * W], f16, tag="yt")
                for half in range(2):
                    ps = ps2_pool.tile([P, 2 * W], f32, tag=f"ps2{half}")
                    for ii in range(2):
                        i = 2 * half + ii
                        for j in range(NBLK):
                            w0, wid = _WINDOWS[j]
                            nc.tensor.matmul(
                                ps[:, ii * W + w0:ii * W + w0 + wid],
                                mids[j // 2][:, (j % 2) * H + P * i:
                                             (j % 2) * H + P * (i + 1)],
                                b2[j][:],
                                start=(j == 0), stop=(j == NBLK - 1),
                            )
                    # DVE takes evac2A plus the front of evac2B; ACT the rest
                    if half == 0:
                        nc.vector.tensor_copy(
                            yt[:, 0:2 * W], ps[:])
                    else:
                        nc.vector.tensor_copy(
                            yt[:, 2 * W:2 * W + 256], ps[:, 0:256])
                        nc.scalar.copy(
                            yt[:, 2 * W + 256:4 * W], ps[:, 256:2 * W])
                nc.sync.dma_start(
                    y[c, :, :].rearrange("(i p) w -> p i w", p=P),
                    yt[:].rearrange("p (i w) -> p i w", w=W))

            mids_prev = None
            for c in range(n_images):
                mids_c = pass1(c)
                if mids_prev is not None:
                    pass2(c - 1, mids_prev)
                mids_prev = mids_c
            pass2(n_images - 1, mids_prev)

    nc.compile()
    return nc


def _run_i8(inputs: dict, trace: bool = False):
    from concourse.bass_utils import run_bass_kernel_spmd

    xf = np.asarray(inputs["input"], dtype=np.float32)
    ker = np.asarray(inputs["kernel"], dtype=np.float32)
    scale = float(ker[0, 0, 0])

    alpha = float(np.abs(xf).max()) / 127.0
    xq = np.clip(np.rint(xf * (1.0 / alpha)), -127, 127).astype(np.int8)

    m = _band_matrix()

    def win(k):
        return m[P * k:P * (k + 1),
                 _WINDOWS[k][0]:_WINDOWS[k][0] + _WINDOWS[k][1]]

    bands1 = [np.ascontiguousarray(win(k) * alpha).astype(np.float16)
              for k in range(NBLK)]
    bands2 = [np.ascontiguousarray(win(k) * scale).astype(np.float16)
              for k in range(NBLK)]

    nc = _build_nc_i8(C)
    in_maps = []
    for b in range(B):
        im = {"x": xq[b]}
        for k in range(NBLK):
            im[f"band1_{k}"] = bands1[k]
            im[f"band2_{k}"] = bands2[k]
        in_maps.append(im)

    res = run_bass_kernel_spmd(nc, in_maps, core_ids=list(range(N_CORES)),
                               trace=trace)
    out = np.stack(
        [res.results[b]["y"].astype(np.float32) for b in range(B)], axis=0)
    return out, res


def _run_f16(inputs: dict, trace: bool = False):
    from concourse.bass_utils import run_bass_kernel_spmd

    x16 = np.asarray(inputs["input"], dtype=np.float32).astype(np.float16)
    ker = np.asarray(inputs["kernel"], dtype=np.float32)
    scale = float(ker[0, 0, 0])

    m = _band_matrix()
    bands1 = [
        np.ascontiguousarray(
            m[P * k:P * (k + 1),
              _WINDOWS[k][0]:_WINDOWS[k][0] + _WINDOWS[k][1]]
        ).astype(np.float16)
        for k in range(NBLK)
    ]
    bands2 = [
        np.ascontiguousarray(
            m[P * k:P * (k + 1),
              _WINDOWS[k][0]:_WINDOWS[k][0] + _WINDOWS[k][1]] * scale
        ).astype(np.float16)
        for k in range(NBLK)
    ]

    nc = _build_nc_f16(C)
    in_maps = []
    for b in range(B):
        im = {"x": x16[b]}
        for k in range(NBLK):
            im[f"band1_{k}"] = bands1[k]
            im[f"band2_{k}"] = bands2[k]
        in_maps.append(im)

    res = run_bass_kernel_spmd(nc, in_maps, core_ids=list(range(N_CORES)),
                               trace=trace)
    out = np.stack(
        [res.results[b]["y"].astype(np.float32) for b in range(B)], axis=0)
    return out, res


# ---------------------------------------------------------------------------
# legacy f32 / mixed / f32r path (previous baseline)
# ---------------------------------------------------------------------------

def _build_nc(scale: float, n_images: int, mode: str):
    import concourse.bacc as bacc
    import concourse.mybir as mybir
    from concourse.tile import TileContext

    f32r = mode == "f32r"
    mixed = mode == "mixed"
    dt = mybir.dt.float32r if f32r else mybir.dt.float32
    wins = _WINDOWS_F32R if f32r else _WINDOWS
    nc = bacc.Bacc(trn_type="TRN2")

    x = nc.dram_tensor("x", [n_images, H, W], dt, kind="ExternalInput")
    band = [
        nc.dram_tensor(f"band{k}", [P, wins[k][1]], dt, kind="ExternalInput")
        for k in range(NBLK)
    ]
    if mixed:
        band_bf = [
            nc.dram_tensor(f"bandbf{k}", [P, wins[k][1]], mybir.dt.bfloat16,
                           kind="ExternalInput")
            for k in range(NBLK)
        ]
    y = nc.dram_tensor("y", [n_images, H, W], mybir.dt.float32,
                       kind="ExternalOutput")

    with TileContext(nc) as tc:
        with (
            tc.tile_pool(name="const", bufs=1) as const_pool,
            tc.tile_pool(name="xin", bufs=12) as x_pool,
            tc.tile_pool(name="mid", bufs=12) as mid_pool,
            tc.tile_pool(name="oout", bufs=12) as out_pool,
            tc.tile_pool(name="ps1", bufs=4, space="PSUM") as ps1_pool,
            tc.tile_pool(name="ps2", bufs=4, space="PSUM") as ps2_pool,
        ):
            band_t = []
            for k in range(NBLK):
                bt = const_pool.tile([P, wins[k][1]], dt, tag=f"band{k}")
                nc.sync.dma_start(bt[:], band[k][:])
                band_t.append(bt)
            band_bf_t = []
            if mixed:
                for k in range(NBLK):
                    bt = const_pool.tile([P, wins[k][1]], mybir.dt.bfloat16,
                                         tag=f"bandbf{k}")
                    nc.sync.dma_start(bt[:], band_bf[k][:])
                    band_bf_t.append(bt)

            def pass1(c):
                # load image as 4 row-strips
                xs = []
                for k in range(NBLK):
                    xt = x_pool.tile([P, W], dt)
                    nc.sync.dma_start(xt[:], x[c, P * k:P * (k + 1), :])
                    xs.append(xt)

                # pass 1: Y1t_j[w, h'] = sum_h X[h, 128j + w] M[h, h']
                y1 = []
                for j in range(NBLK):
                    ps = ps1_pool.tile([P, H], mybir.dt.float32)
                    for k in range(NBLK):
                        w0, wid = wins[k]
                        nc.tensor.matmul(
                            ps[:, w0:w0 + wid],
                            xs[k][:, P * j:P * (j + 1)],
                            band_t[k][:],
                            start=(k == 0), stop=(k == NBLK - 1),
                        )
                    if mixed:
                        # evacuate as bf16 hi + bf16 lo (exact to ~2^-18);
                        # bank 0 chains hi+lo on DVE (no cross-engine hop) so
                        # pass2 can start earliest; other banks' hi go to ACT
                        hi = mid_pool.tile([P, H], mybir.dt.bfloat16, tag="hi")
                        lo = mid_pool.tile([P, H], mybir.dt.bfloat16, tag="lo")
                        if j == 0:
                            nc.vector.tensor_copy(hi[:], ps[:])
                        else:
                            nc.scalar.copy(hi[:], ps[:])
                        nc.vector.tensor_sub(lo[:], ps[:], hi[:])
                        y1.append((hi, lo))
                    else:
                        yt = mid_pool.tile([P, H], dt)
                        if j % 2 == 0:
                            nc.vector.tensor_copy(yt[:], ps[:])
                        else:
                            nc.scalar.copy(yt[:], ps[:])
                        y1.append(yt)
                return y1

            def pass2(c, y1):
                # pass 2: out_i[h', w'] = sum_w Y1t[w, 128i + h'] M[w, w']
                for i in range(NBLK):
                    ps = ps2_pool.tile([P, W], mybir.dt.float32, name="ps2",
                                       tag="ps2")
                    for j in range(NBLK):
                        w0, wid = wins[j]
                        if mixed:
                            hi, lo = y1[j]
                            nc.tensor.matmul(
                                ps[:, w0:w0 + wid],
                                hi[:, P * i:P * (i + 1)],
                                band_bf_t[j][:],
                                start=(j == 0), stop=False,
                            )
                            nc.tensor.matmul(
                                ps[:, w0:w0 + wid],
                                lo[:, P * i:P * (i + 1)],
                                band_bf_t[j][:],
                                start=False, stop=(j == NBLK - 1),
                            )
                        else:
                            nc.tensor.matmul(
                                ps[:, w0:w0 + wid],
                                y1[j][:, P * i:P * (i + 1)],
                                band_t[j][:],
                                start=(j == 0), stop=(j == NBLK - 1),
                            )
                    ot = out_pool.tile([P, W], mybir.dt.float32)
                    if (not mixed and i % 2 == 0) or (mixed and i == 0):
                        nc.vector.tensor_scalar_mul(ot[:], ps[:], scale)
                    else:
                        nc.scalar.mul(ot[:], ps[:], scale)
                    nc.sync.dma_start(y[c, P * i:P * (i + 1), :], ot[:])

            for c in range(n_images):
                pass2(c, pass1(c))

    nc.compile()
    return nc


def _run(inputs: dict, mode: str = "f16", trace: bool = False):
    if mode == "f16":
        return _run_f16(inputs, trace=trace)
    if mode == "i8":
        return _run_i8(inputs, trace=trace)

    import ml_dtypes
    from concourse.bass_utils import run_bass_kernel_spmd

    x = np.ascontiguousarray(inputs["input"], dtype=np.float32)
    ker = np.asarray(inputs["kernel"], dtype=np.float32)
    scale = float(ker[0, 0, 0])

    wins = _WINDOWS_F32R if mode == "f32r" else _WINDOWS
    m = _band_matrix()
    bands = [
        np.ascontiguousarray(m[P * k:P * (k + 1), wins[k][0]:wins[k][0] + wins[k][1]])
        for k in range(NBLK)
    ]

    nc = _build_nc(scale, C, mode)
    in_maps = []
    for b in range(B):
        im = {"x": x[b]}
        for k in range(NBLK):
            im[f"band{k}"] = bands[k]
            if mode == "mixed":
                im[f"bandbf{k}"] = bands[k].astype(ml_dtypes.bfloat16)
        in_maps.append(im)

    res = run_bass_kernel_spmd(nc, in_maps, core_ids=list(range(N_CORES)),
                               trace=trace)
    out = np.stack([res.results[b]["y"] for b in range(B)], axis=0)
    return out, res


def kernel(**inputs) -> np.ndarray:
    out, _ = _run(inputs)
    return out


# revision 13
# speedup vs baseline: 2.2377x; 1.1146x over previous
"""BoxBlur 13x13 depthwise conv (reflect pad) on 8 trn2 NeuronCores.

Input (8, 64, 512, 512) f32 + kernel (1, 13, 13) f32 -> output (8, 64, 512, 512).

Sharding: batch dim across 8 cores (one sample = 64 channel-images per core).

Algorithm (per 512x512 image): box blur is separable. Both 1D 13-tap passes
(reflect padding folded into an integer band matrix M[h, h'] built on host)
run on the tensor engine as normal-mode matmuls with the image block as the
STATIONARY operand and the band matrix as the MOVING operand, which fuses a
transpose into each pass:

    pass1:  Y1t[w, h'] = sum_h X[h, w] * M[h, h']      (vconv, output transposed)
    pass2:  out[h', w'] = sum_w Y1t[w, h'] * M2[w, w'] (hconv, transpose undone)

Each pass is 4 contraction blocks x 4 stationary 128-slices = 16 matmuls per
image, PSUM-accumulated over the contraction blocks using partial-range
windows (the band is zero outside a ~140-wide window per block).

Default mode "f16": everything fp16 (the rel-err budget is 2e-2; fp16
end-to-end lands ~3e-4). This halves HBM traffic (the bottleneck) vs f32 and
runs every matmul single-pass at 1 cyc/row. The host converts in/out; the
1/169 scale is folded into the pass-2 band so every PSUM evacuation is a
plain copy. IO is one batched DMA per image per direction ([128, 2048]
slabs via rearranged access patterns) to stay under the SP DGE issue rate,
and PSUM is split into 2-bank tiles so evacuations are 2 big copies per pass
balanced across ACT and DVE.

Legacy modes kept for reference: f32 / mixed / f32r (see _build_nc).
"""
import numpy as np

B, C, H, W = 8, 64, 512, 512
KY = KX = 13
HALF = 6
N_CORES = 8
P = 128
NBLK = H // P  # 4

# per contraction block k: window [start, width) of nonzero band columns
_WINDOWS = [
    (max(0, P * k - HALF),
     min(H, P * k + P - 1 + HALF + 1) - max(0, P * k - HALF))
    for k in range(NBLK)
]
# fp32r runs at 1 cyc/row only when the moving free dim is >= 256: use
# widened 256-col windows (zero band entries outside the true span are
# harmless -- PSUM accumulate/overwrite-by-has_written keeps them exact)
_WINDOWS_F32R = [
    (min(max(0, P * k - HALF), H - 256), 256)
    for k in range(NBLK)
]


def _band_matrix() -> np.ndarray:
    """M[h, h'] = number of taps of output h' that hit input row h
    (13-tap, reflect padding, pad = 6 both sides)."""
    m = np.zeros((H, H), dtype=np.float32)
    for hp in range(H):
        for d in range(-HALF, HALF + 1):
            h = hp + d
            if h < 0:
                h = -h
            if h > H - 1:
                h = 2 * (H - 1) - h
            m[h, hp] += 1.0
    return m


def _build_nc_f16(n_images: int):
    import concourse.bacc as bacc
    import concourse.mybir as mybir
    from concourse.tile import TileContext

    f16 = mybir.dt.float16
    f32 = mybir.dt.float32
    nc = bacc.Bacc(trn_type="TRN2")

    x = nc.dram_tensor("x", [n_images, H, W], f16, kind="ExternalInput")
    band1 = [
        nc.dram_tensor(f"band1_{k}", [P, _WINDOWS[k][1]], f16,
                       kind="ExternalInput")
        for k in range(NBLK)
    ]
    band2 = [
        nc.dram_tensor(f"band2_{k}", [P, _WINDOWS[k][1]], f16,
                       kind="ExternalInput")
        for k in range(NBLK)
    ]
    y = nc.dram_tensor("y", [n_images, H, W], f16, kind="ExternalOutput")

    with TileContext(nc) as tc:
        with (
            tc.tile_pool(name="const", bufs=1) as const_pool,
            tc.tile_pool(name="xin", bufs=10) as x_pool,
            tc.tile_pool(name="mid", bufs=4) as mid_pool,
            tc.tile_pool(name="yout", bufs=6) as y_pool,
            tc.tile_pool(name="ps1", bufs=1, space="PSUM") as ps1_pool,
            tc.tile_pool(name="ps2", bufs=1, space="PSUM") as ps2_pool,
        ):
            b1, b2 = [], []
            for k in range(NBLK):
                t1 = const_pool.tile([P, _WINDOWS[k][1]], f16, tag=f"b1{k}")
                nc.sync.dma_start(t1[:], band1[k][:])
                b1.append(t1)
                t2 = const_pool.tile([P, _WINDOWS[k][1]], f16, tag=f"b2{k}")
                nc.sync.dma_start(t2[:], band2[k][:])
                b2.append(t2)

            # per-image IO slabs (GI=2 was tried and regressed: coarser DMA
            # granularity couples the pipeline and lengthens the chains)
            GI = 1  # images per IO slab
            IMG = NBLK * W  # 2048 cols per image

            def pass1(c, xt):
                # per half = two j-slices into one 2-bank PSUM tile
                xoff = (c % GI) * IMG
                mids = []
                for half in range(2):
                    ps = ps1_pool.tile([P, 2 * H], f32, tag=f"ps1{half}")
                    for jj in range(2):
                        j = 2 * half + jj
                        for k in range(NBLK):
                            w0, wid = _WINDOWS[k]
                            nc.tensor.matmul(
                                ps[:, jj * H + w0:jj * H + w0 + wid],
                                xt[:, xoff + k * W + P * j:
                                   xoff + k * W + P * (j + 1)],
                                b1[k][:],
                                start=(k == 0), stop=(k == NBLK - 1),
                            )
                    mt = mid_pool.tile([P, 2 * H], f16, tag=f"mid{half}")
                    if half == 0:
                        nc.scalar.copy(mt[:], ps[:])
                    else:
                        nc.vector.tensor_copy(mt[:], ps[:])
                    mids.append(mt)
                return mids

            def pass2(c, mids, yt):
                # per half = two i-slices into one 2-bank PSUM tile
                yoff = (c % GI) * IMG
                for half in range(2):
                    ps = ps2_pool.tile([P, 2 * W], f32, tag=f"ps2{half}")
                    for ii in range(2):
                        i = 2 * half + ii
                        for j in range(NBLK):
                            w0, wid = _WINDOWS[j]
                            nc.tensor.matmul(
                                ps[:, ii * W + w0:ii * W + w0 + wid],
                                mids[j // 2][:, (j % 2) * H + P * i:
                                             (j % 2) * H + P * (i + 1)],
                                b2[j][:],
                                start=(j == 0), stop=(j == NBLK - 1),
                            )
                    if half == 0:
                        nc.vector.tensor_copy(
                            yt[:, yoff + half * 2 * W:
                               yoff + (half + 1) * 2 * W], ps[:])
                    else:
                        nc.scalar.copy(
                            yt[:, yoff + half * 2 * W:
                               yoff + (half + 1) * 2 * W], ps[:])
                if c % GI == GI - 1:
                    c0 = c - (GI - 1)
                    # SWDGE (GpSimd) queue: keeps output stores off the SP
                    # HWDGE FIFO so they never block input prefetch
                    nc.gpsimd.dma_start(
                        y[c0:c0 + GI, :, :].rearrange(
                            "n (i p) w -> p n i w", p=P),
                        yt[:].rearrange("p (n i w) -> p n i w", n=GI, w=W))

            def load_slab(c0):
                xt = x_pool.tile([P, GI * IMG], f16, tag="xt")
                nc.sync.dma_start(
                    xt[:].rearrange("p (n k w) -> p n k w", n=GI, w=W),
                    x[c0:c0 + GI, :, :].rearrange("n (k p) w -> p n k w", p=P))
                return xt

            # software pipeline: pass2 runs one image behind pass1, so the
            # PE fills the mid-evacuation latency of image c with pass-1
            # matmuls of image c+1 instead of stalling
            mids_prev = None
            xt_cur = None
            yt_cur = None
            yt_prev = None
            for c in range(n_images):
                if c % GI == 0:
                    xt_cur = load_slab(c)
                mids_c = pass1(c, xt_cur)
                if mids_prev is not None:
                    cp = c - 1
                    if cp % GI == 0:
                        yt_prev = y_pool.tile([P, GI * IMG], f16, tag="yt")
                    pass2(cp, mids_prev, yt_prev)
                mids_prev = mids_c
            cp = n_images - 1
            if cp % GI == 0:
                yt_prev = y_pool.tile([P, GI * IMG], f16, tag="yt")
            pass2(cp, mids_prev, yt_prev)

    nc.compile()
    return nc


def _build_nc_i8(n_images: int):
    """int8-input variant: input DMA'd as int8 (halves input HBM traffic),
    upcast to f16 on the GpSimd engine (otherwise idle), dequant scale folded
    into the pass-1 band. Output stays f16. IO DMAs on the SP HWDGE queue;
    GpSimd is dedicated to the casts."""
    import concourse.bacc as bacc
    import concourse.mybir as mybir
    from concourse.tile import TileContext

    i8 = mybir.dt.int8
    f16 = mybir.dt.float16
    f32 = mybir.dt.float32
    nc = bacc.Bacc(trn_type="TRN2")

    x = nc.dram_tensor("x", [n_images, H, W], i8, kind="ExternalInput")
    band1 = [
        nc.dram_tensor(f"band1_{k}", [P, _WINDOWS[k][1]], f16,
                       kind="ExternalInput")
        for k in range(NBLK)
    ]
    band2 = [
        nc.dram_tensor(f"band2_{k}", [P, _WINDOWS[k][1]], f16,
                       kind="ExternalInput")
        for k in range(NBLK)
    ]
    y = nc.dram_tensor("y", [n_images, H, W], f16, kind="ExternalOutput")

    with TileContext(nc) as tc:
        with (
            tc.tile_pool(name="const", bufs=1) as const_pool,
            tc.tile_pool(name="xq", bufs=10) as xq_pool,
            tc.tile_pool(name="xin", bufs=4) as x_pool,
            tc.tile_pool(name="mid", bufs=4) as mid_pool,
            tc.tile_pool(name="yout", bufs=6) as y_pool,
            tc.tile_pool(name="ps1", bufs=1, space="PSUM") as ps1_pool,
            tc.tile_pool(name="ps2", bufs=1, space="PSUM") as ps2_pool,
        ):
            b1, b2 = [], []
            for k in range(NBLK):
                t1 = const_pool.tile([P, _WINDOWS[k][1]], f16, tag=f"b1{k}")
                nc.sync.dma_start(t1[:], band1[k][:])
                b1.append(t1)
                t2 = const_pool.tile([P, _WINDOWS[k][1]], f16, tag=f"b2{k}")
                nc.sync.dma_start(t2[:], band2[k][:])
                b2.append(t2)

            def pass1(c):
                xq = xq_pool.tile([P, NBLK * W], i8, tag="xq")
                nc.sync.dma_start(
                    xq[:].rearrange("p (k w) -> p k w", w=W),
                    x[c, :, :].rearrange("(k p) w -> p k w", p=P))
                # upcast int8 -> f16 on the idle GpSimd engine
                xt = x_pool.tile([P, NBLK * W], f16, tag="xt")
                nc.gpsimd.tensor_copy(xt[:], xq[:])

                mids = []
                for half in range(2):
                    ps = ps1_pool.tile([P, 2 * H], f32, tag=f"ps1{half}")
                    for jj in range(2):
                        j = 2 * half + jj
                        for k in range(NBLK):
                            w0, wid = _WINDOWS[k]
                            nc.tensor.matmul(
                                ps[:, jj * H + w0:jj * H + w0 + wid],
                                xt[:, k * W + P * j:k * W + P * (j + 1)],
                                b1[k][:],
                                start=(k == 0), stop=(k == NBLK - 1),
                            )
                    mt = mid_pool.tile([P, 2 * H], f16, tag=f"mid{half}")
                    if half == 0:
                        nc.scalar.copy(mt[:], ps[:])
                    else:
                        nc.vector.tensor_copy(mt[:], ps[:])
                    mids.append(mt)
                return mids

            def pass2(c, mids):
                yt = y_pool.tile([P, NBLK * W], f16, tag="yt")
                for half in range(2):
                    ps = ps2_pool.tile([P, 2 * W], f32, tag=f"ps2{half}")
                    for ii in range(2):
                        i = 2 * half + ii
                        for j in range(NBLK):
                            w0, wid = _WINDOWS[j]
                            nc.tensor.matmul(
                                ps[:, ii * W + w0:ii * W + w0 + wid],
                                mids[j // 2][:, (j % 2) * H + P * i:
                                             (j % 2) * H + P * (i + 1)],
                                b2[j][:],
                                start=(j == 0), stop=(j == NBLK - 1),
                            )
                    # DVE takes evac2A plus the front of evac2B; ACT the rest
                    if half == 0:
                        nc.vector.tensor_copy(
                            yt[:, 0:2 * W], ps[:])
                    else:
                        nc.vector.tensor_copy(
                            yt[:, 2 * W:2 * W + 256], ps[:, 0:256])
                        nc.scalar.copy(
                            yt[:, 2 * W + 256:4 * W], ps[:, 256:2 * W])
                nc.sync.dma_start(
                    y[c, :, :].rearrange("(i p) w -> p i w", p=P),
                    yt[:].rearrange("p (i w) -> p i w", w=W))

            mids_prev = None
            for c in range(n_images):
                mids_c = pass1(c)
                if mids_prev is not None:
                    pass2(c - 1, mids_prev)
                mids_prev = mids_c
            pass2(n_images - 1, mids_prev)

    nc.compile()
    return nc


def _run_i8(inputs: dict, trace: bool = False):
    from concourse.bass_utils import run_bass_kernel_spmd

    xf = np.asarray(inputs["input"], dtype=np.float32)
    ker = np.asarray(inputs["kernel"], dtype=np.float32)
    scale = float(ker[0, 0, 0])

    alpha = float(np.abs(xf).max()) / 127.0
    xq = np.clip(np.rint(xf * (1.0 / alpha)), -127, 127).astype(np.int8)

    m = _band_matrix()

    def win(k):
        return m[P * k:P * (k + 1),
                 _WINDOWS[k][0]:_WINDOWS[k][0] + _WINDOWS[k][1]]

    bands1 = [np.ascontiguousarray(win(k) * alpha).astype(np.float16)
              for k in range(NBLK)]
    bands2 = [np.ascontiguousarray(win(k) * scale).astype(np.float16)
              for k in range(NBLK)]

    nc = _build_nc_i8(C)
    in_maps = []
    for b in range(B):
        im = {"x": xq[b]}
        for k in range(NBLK):
            im[f"band1_{k}"] = bands1[k]
            im[f"band2_{k}"] = bands2[k]
        in_maps.append(im)

    res = run_bass_kernel_spmd(nc, in_maps, core_ids=list(range(N_CORES)),
                               trace=trace)
    out = np.stack(
        [res.results[b]["y"].astype(np.float32) for b in range(B)], axis=0)
    return out, res


def _run_f16(inputs: dict, trace: bool = False):
    from concourse.bass_utils import run_bass_kernel_spmd

    x16 = np.asarray(inputs["input"], dtype=np.float32).astype(np.float16)
    ker = np.asarray(inputs["kernel"], dtype=np.float32)
    scale = float(ker[0, 0, 0])

    m = _band_matrix()
    bands1 = [
        np.ascontiguousarray(
            m[P * k:P * (k + 1),
              _WINDOWS[k][0]:_WINDOWS[k][0] + _WINDOWS[k][1]]
        ).astype(np.float16)
        for k in range(NBLK)
    ]
    bands2 = [
        np.ascontiguousarray(
            m[P * k:P * (k + 1),
              _WINDOWS[k][0]:_WINDOWS[k][0] + _WINDOWS[k][1]] * scale
        ).astype(np.float16)
        for k in range(NBLK)
    ]

    nc = _build_nc_f16(C)
    in_maps = []
    for b in range(B):
        im = {"x": x16[b]}
        for k in range(NBLK):
            im[f"band1_{k}"] = bands1[k]
            im[f"band2_{k}"] = bands2[k]
        in_maps.append(im)

    res = run_bass_kernel_spmd(nc, in_maps, core_ids=list(range(N_CORES)),
                               trace=trace)
    out = np.stack(
        [res.results[b]["y"].astype(np.float32) for b in range(B)], axis=0)
    return out, res


# ---------------------------------------------------------------------------
# legacy f32 / mixed / f32r path (previous baseline)
# ---------------------------------------------------------------------------

def _build_nc(scale: float, n_images: int, mode: str):
    import concourse.bacc as bacc
    import concourse.mybir as mybir
    from concourse.tile import TileContext

    f32r = mode == "f32r"
    mixed = mode == "mixed"
    dt = mybir.dt.float32r if f32r else mybir.dt.float32
    wins = _WINDOWS_F32R if f32r else _WINDOWS
    nc = bacc.Bacc(trn_type="TRN2")

    x = nc.dram_tensor("x", [n_images, H, W], dt, kind="ExternalInput")
    band = [
        nc.dram_tensor(f"band{k}", [P, wins[k][1]], dt, kind="ExternalInput")
        for k in range(NBLK)
    ]
    if mixed:
        band_bf = [
            nc.dram_tensor(f"bandbf{k}", [P, wins[k][1]], mybir.dt.bfloat16,
                           kind="ExternalInput")
            for k in range(NBLK)
        ]
    y = nc.dram_tensor("y", [n_images, H, W], mybir.dt.float32,
                       kind="ExternalOutput")

    with TileContext(nc) as tc:
        with (
            tc.tile_pool(name="const", bufs=1) as const_pool,
            tc.tile_pool(name="xin", bufs=12) as x_pool,
            tc.tile_pool(name="mid", bufs=12) as mid_pool,
            tc.tile_pool(name="oout", bufs=12) as out_pool,
            tc.tile_pool(name="ps1", bufs=4, space="PSUM") as ps1_pool,
            tc.tile_pool(name="ps2", bufs=4, space="PSUM") as ps2_pool,
        ):
            band_t = []
            for k in range(NBLK):
                bt = const_pool.tile([P, wins[k][1]], dt, tag=f"band{k}")
                nc.sync.dma_start(bt[:], band[k][:])
                band_t.append(bt)
            band_bf_t = []
            if mixed:
                for k in range(NBLK):
                    bt = const_pool.tile([P, wins[k][1]], mybir.dt.bfloat16,
                                         tag=f"bandbf{k}")
                    nc.sync.dma_start(bt[:], band_bf[k][:])
                    band_bf_t.append(bt)

            def pass1(c):
                # load image as 4 row-strips
                xs = []
                for k in range(NBLK):
                    xt = x_pool.tile([P, W], dt)
                    nc.sync.dma_start(xt[:], x[c, P * k:P * (k + 1), :])
                    xs.append(xt)

                # pass 1: Y1t_j[w, h'] = sum_h X[h, 128j + w] M[h, h']
                y1 = []
                for j in range(NBLK):
                    ps = ps1_pool.tile([P, H], mybir.dt.float32)
                    for k in range(NBLK):
                        w0, wid = wins[k]
                        nc.tensor.matmul(
                            ps[:, w0:w0 + wid],
                            xs[k][:, P * j:P * (j + 1)],
                            band_t[k][:],
                            start=(k == 0), stop=(k == NBLK - 1),
                        )
                    if mixed:
                        # evacuate as bf16 hi + bf16 lo (exact to ~2^-18);
                        # bank 0 chains hi+lo on DVE (no cross-engine hop) so
                        # pass2 can start earliest; other banks' hi go to ACT
                        hi = mid_pool.tile([P, H], mybir.dt.bfloat16, tag="hi")
                        lo = mid_pool.tile([P, H], mybir.dt.bfloat16, tag="lo")
                        if j == 0:
                            nc.vector.tensor_copy(hi[:], ps[:])
                        else:
                            nc.scalar.copy(hi[:], ps[:])
                        nc.vector.tensor_sub(lo[:], ps[:], hi[:])
                        y1.append((hi, lo))
                    else:
                        yt = mid_pool.tile([P, H], dt)
                        if j % 2 == 0:
                            nc.vector.tensor_copy(yt[:], ps[:])
                        else:
                            nc.scalar.copy(yt[:], ps[:])
                        y1.append(yt)
                return y1

            def pass2(c, y1):
                # pass 2: out_i[h', w'] = sum_w Y1t[w, 128i + h'] M[w, w']
                for i in range(NBLK):
                    ps = ps2_pool.tile([P, W], mybir.dt.float32, name="ps2",
                                       tag="ps2")
                    for j in range(NBLK):
                        w0, wid = wins[j]
                        if mixed:
                            hi, lo = y1[j]
                            nc.tensor.matmul(
                                ps[:, w0:w0 + wid],
                                hi[:, P * i:P * (i + 1)],
                                band_bf_t[j][:],
                                start=(j == 0), stop=False,
                            )
                            nc.tensor.matmul(
                                ps[:, w0:w0 + wid],
                                lo[:, P * i:P * (i + 1)],
                                band_bf_t[j][:],
                                start=False, stop=(j == NBLK - 1),
                            )
                        else:
                            nc.tensor.matmul(
                                ps[:, w0:w0 + wid],
                                y1[j][:, P * i:P * (i + 1)],
                                band_t[j][:],
                                start=(j == 0), stop=(j == NBLK - 1),
                            )
                    ot = out_pool.tile([P, W], mybir.dt.float32)
                    if (not mixed and i % 2 == 0) or (mixed and i == 0):
                        nc.vector.tensor_scalar_mul(ot[:], ps[:], scale)
                    else:
                        nc.scalar.mul(ot[:], ps[:], scale)
                    nc.sync.dma_start(y[c, P * i:P * (i + 1), :], ot[:])

            for c in range(n_images):
                pass2(c, pass1(c))

    nc.compile()
    return nc


def _run(inputs: dict, mode: str = "f16", trace: bool = False):
    if mode == "f16":
        return _run_f16(inputs, trace=trace)
    if mode == "i8":
        return _run_i8(inputs, trace=trace)

    import ml_dtypes
    from concourse.bass_utils import run_bass_kernel_spmd

    x = np.ascontiguousarray(inputs["input"], dtype=np.float32)
    ker = np.asarray(inputs["kernel"], dtype=np.float32)
    scale = float(ker[0, 0, 0])

    wins = _WINDOWS_F32R if mode == "f32r" else _WINDOWS
    m = _band_matrix()
    bands = [
        np.ascontiguousarray(m[P * k:P * (k + 1), wins[k][0]:wins[k][0] + wins[k][1]])
        for k in range(NBLK)
    ]

    nc = _build_nc(scale, C, mode)
    in_maps = []
    for b in range(B):
        im = {"x": x[b]}
        for k in range(NBLK):
            im[f"band{k}"] = bands[k]
            if mode == "mixed":
                im[f"bandbf{k}"] = bands[k].astype(ml_dtypes.bfloat16)
        in_maps.append(im)

    res = run_bass_kernel_spmd(nc, in_maps, core_ids=list(range(N_CORES)),
                               trace=trace)
    out = np.stack([res.results[b]["y"] for b in range(B)], axis=0)
    return out, res


def kernel(**inputs) -> np.ndarray:
    out, _ = _run(inputs)
    return out
